# revision 1
# baseline (speedup 1.0000x reference)
"""Performer attention (FAVOR+) sharded across 8 Trainium2 NeuronCores.

Sharding: 8 cores = 4 batches x 2 head-groups (6 heads each).  Every core
computes LayerNorm + its slice of the qkv projection, the FAVOR+ feature
maps, linear-attention aggregation, and a partial output projection.  The
k-feature global max is exact via a cross-core max all-reduce (lax.pmax).
Host sums the two partial projections per batch and adds the v-residual.

Falls back to an exact numpy implementation if the device path fails.
"""
import numpy as np

EPS = 1e-8
LN_EPS = 1e-5
H = 12
DH = 64
M = 384
EMB = 768
NUM_REAL_SQRT = 8.0
B, N, C = 4, 3136, 768
HG = 2            # head groups
HPG = H // HG     # heads per group (6)
SL = HPG * DH     # slice width per head-group (384)


def _kernel_device(x, ln_w, ln_b, qkv_w, qkv_b, proj_w, proj_b, w):
    import jax
    import jax.numpy as jnp

    try:
        devs = jax.devices("axon")
    except Exception:
        devs = [d for d in jax.devices() if d.platform != "cpu"]
    assert len(devs) >= 8, f"need 8 devices, got {len(devs)}"

    normal2_half = 0.5 / np.sqrt(np.sqrt(DH)) ** 2   # 0.5 * (DH**-0.25)**2
    ratio = 1.0 / M ** 0.25

    def core_fn(xb, wqkv_s, bqkv_s, proj_s, lnw, lnb, wmat):
        # xb: (N, C)  wqkv_s: (3*SL, C)  bqkv_s: (3*SL,)  proj_s: (C, SL)
        mu = xb.mean(-1, keepdims=True)
        var = jnp.var(xb, axis=-1, keepdims=True)
        h = (xb - mu) * jax.lax.rsqrt(var + LN_EPS) * lnw + lnb
        qkv = h @ wqkv_s.T + bqkv_s                      # (N, 3*SL)
        qkv = qkv.reshape(N, 3, HPG, DH)
        q = qkv[:, 0].transpose(1, 0, 2)                 # (HPG, N, DH)
        k = qkv[:, 1].transpose(1, 0, 2)
        v = qkv[:, 2].transpose(1, 0, 2)

        dash_k = jnp.einsum('hnc,mc->hnm', k, wmat)      # (HPG, N, M)
        diag_k = (jnp.square(k).sum(-1) * normal2_half)[..., None]
        mx = jax.lax.pmax(jnp.max(dash_k), axis_name='i')   # exact global max
        kp = ratio * (jnp.exp(dash_k - diag_k - mx) + EPS)

        dash_q = jnp.einsum('hnc,mc->hnm', q, wmat)
        diag_q = (jnp.square(q).sum(-1) * normal2_half)[..., None]
        mxq = jnp.max(dash_q, axis=-1, keepdims=True)
        qp = ratio * (jnp.exp(dash_q - diag_q - mxq) + EPS)

        Dn = jnp.einsum('hnm,hm->hn', qp, kp.sum(1))[..., None]
        kptv = jnp.einsum('hnd,hnm->hdm', v, kp)
        y = jnp.einsum('hnm,hdm->hnd', qp, kptv)
        y = y / (Dn + EPS)
        y_flat = y.transpose(1, 0, 2).reshape(N, SL) / NUM_REAL_SQRT
        partial = y_flat @ proj_s.T                      # (N, C)
        vf_own = v.transpose(1, 0, 2).reshape(N, SL)
        return partial, vf_own

    # Build per-core inputs: core c = (b, hg) with b = c // 2, hg = c % 2
    xs, wq, bq, ps = [], [], [], []
    for c in range(8):
        b, hg = divmod(c, HG)
        xs.append(x[b])
        rows = np.concatenate([
            qkv_w[s * EMB + hg * SL: s * EMB + (hg + 1) * SL] for s in range(3)
        ], axis=0)                                       # (3*SL, C)
        wq.append(rows)
        bq.append(np.concatenate([
            qkv_b[s * EMB + hg * SL: s * EMB + (hg + 1) * SL] for s in range(3)
        ]))
        ps.append(proj_w[:, hg * SL:(hg + 1) * SL])      # (C, SL)
    xs = np.stack(xs); wq = np.stack(wq); bq = np.stack(bq); ps = np.stack(ps)
    rep = lambda a: np.broadcast_to(a, (8,) + a.shape).copy()

    pf = jax.pmap(core_fn, axis_name='i', devices=devs[:8])
    partial, vf_own = pf(xs, wq, bq, ps, rep(ln_w), rep(ln_b), rep(w))
    partial = np.asarray(partial, dtype=np.float32)      # (8, N, C)
    vf_own = np.asarray(vf_own, dtype=np.float32)        # (8, N, SL)

    out = np.empty((B, N, C), dtype=np.float32)
    for b in range(B):
        acc = partial[2 * b] + partial[2 * b + 1] + proj_b[None, :]
        acc[:, 0 * SL:1 * SL] += vf_own[2 * b]
        acc[:, 1 * SL:2 * SL] += vf_own[2 * b + 1]
        out[b] = acc
    if not np.all(np.isfinite(out)):
        raise FloatingPointError("non-finite output from device path")
    return out


def _kernel_numpy(x, ln_w, ln_b, qkv_w, qkv_b, proj_w, proj_b, w):
    x = x.astype(np.float32)
    mu = x.mean(-1, keepdims=True, dtype=np.float32)
    var = x.var(-1, keepdims=True, dtype=np.float32)
    h = (x - mu) / np.sqrt(var + LN_EPS) * ln_w + ln_b
    qkv = (h.reshape(B * N, C) @ qkv_w.T + qkv_b).reshape(B, N, 3, H, DH)
    qkv = qkv.transpose(2, 0, 3, 1, 4)                   # (3, B, H, N, DH)
    q, k, v = qkv[0], qkv[1], qkv[2]
    normal2_half = np.float32(0.5 / np.sqrt(DH))
    ratio = np.float32(1.0 / M ** 0.25)

    dash_k = np.einsum('bhnc,mc->bhnm', k, w, optimize=True)
    diag_k = (np.square(k).sum(-1) * normal2_half)[..., None]
    kp = ratio * (np.exp(dash_k - diag_k - dash_k.max()) + np.float32(EPS))
    del dash_k
    dash_q = np.einsum('bhnc,mc->bhnm', q, w, optimize=True)
    diag_q = (np.square(q).sum(-1) * normal2_half)[..., None]
    qp = ratio * (np.exp(dash_q - diag_q - dash_q.max(-1, keepdims=True))
                  + np.float32(EPS))
    del dash_q
    Dn = np.einsum('bhnm,bhm->bhn', qp, kp.sum(2), optimize=True)[..., None]
    kptv = np.einsum('bhnd,bhnm->bhdm', v, kp, optimize=True)
    y = np.einsum('bhnm,bhdm->bhnd', qp, kptv, optimize=True)
    y = y / (Dn + np.float32(EPS))
    y = y.transpose(0, 2, 1, 3).reshape(B, N, EMB) / np.float32(NUM_REAL_SQRT)
    vf = v.transpose(0, 2, 1, 3).reshape(B, N, EMB)
    return (vf + y.reshape(B * N, EMB) @ proj_w.T + proj_b).reshape(B, N, C)


def kernel(x, ln_w, ln_b, qkv_w, qkv_b, proj_w, proj_b, w):
    args = (np.asarray(x, np.float32), np.asarray(ln_w, np.float32),
            np.asarray(ln_b, np.float32), np.asarray(qkv_w, np.float32),
            np.asarray(qkv_b, np.float32), np.asarray(proj_w, np.float32),
            np.asarray(proj_b, np.float32), np.asarray(w, np.float32))
    try:
        return _kernel_device(*args)
    except Exception:
        return _kernel_numpy(*args)

